# revision 11
# baseline (speedup 1.0000x reference)
"""Trainium2 Bass kernel for nn_BlueBoxLayer (RBF-kernel attention + LISTA soft-threshold).

reference:
    DH  = D @ H                          [n=512, T=8192]
    G   = DH^T DH                        [T, T]
    attn= softmax(G + log_beta[None,:], axis=1),  log_beta = -0.5*colsum(DH^2)
    Z   = l2 * (H @ attn)                [d=128, T]
    out = softthresh(U @ Z + V @ X, l1)  [d=128, T]

Strategy: 8-way sequence parallel, flash-style fused attention (the [T,T]
attention matrix is never materialized).  Core m owns token rows
[1024m, 1024(m+1)).  It computes its row block of G chunk-by-chunk on the
tensor engine (fp16 inputs, fp32 PSUM), exponentiates with a per-row
Cauchy-Schwarz-safe offset c_t = 0.5*||DH_t||^2 (so every exponent <= 0),
multiplies by beta_j and row-reduces on the vector engine, and accumulates
a partial Z_m = (l2/r_t scaled H^T)^T @ F into SBUF.  The 8 partial Z's
([128, 8192] each) are ReduceScattered so each core lands exactly its own
column shard of Z, then applies U@Z + V@X and the soft threshold locally.
Host only does input layout prep (dtype casts / transposes / slices) and
output concat.
"""

import os
import sys

for _p in ("/opt/trn_rl_repo", "/root/.axon_site/_ro/trn_rl_repo"):
    if os.path.isdir(_p) and _p not in sys.path:
        sys.path.insert(0, _p)

import numpy as np

import concourse.bass as bass
import concourse.mybir as mybir
import concourse.tile as tile
from concourse import bacc

N_CORES = 8
T = 8192
TS = T // N_CORES  # 1024 tokens per core
DD = 128  # feature dim d
NN = 512  # dictionary dim n
MM = 384  # measurement dim m
NC_JC = T // 512  # 16 column chunks of 512
NC_RT = TS // 128  # 8 row tiles per core

F32 = mybir.dt.float32
F16 = mybir.dt.float16
AF = mybir.ActivationFunctionType
OP = mybir.AluOpType


def _emit(nc, tc, io, thres, nrep, T_=T, stage="full"):
    """Emit the whole program under an open TileContext."""
    import contextlib

    TS_ = T_ // N_CORES
    NJC = T_ // 512
    NRT = TS_ // 128
    NJCS = TS_ // 512  # column chunks per shard

    ctx = contextlib.ExitStack()
    with ctx:
        # ---------------- persistent pools ----------------
        pdh = ctx.enter_context(tc.tile_pool(name="pdh", bufs=4))
        pdhl = ctx.enter_context(tc.tile_pool(name="pdhl", bufs=4))
        pbc = ctx.enter_context(tc.tile_pool(name="pbc", bufs=NJC))
        pz = ctx.enter_context(tc.tile_pool(name="pz", bufs=1))
        pht = ctx.enter_context(tc.tile_pool(name="pht", bufs=NRT))
        pnegc = ctx.enter_context(tc.tile_pool(name="pnegc", bufs=NRT))
        psc = ctx.enter_context(tc.tile_pool(name="psc", bufs=2))
        pG = ctx.enter_context(tc.tile_pool(name="pG", bufs=3, space="PSUM"))
        pZp = ctx.enter_context(tc.tile_pool(name="pZp", bufs=3, space="PSUM"))
        pdram = ctx.enter_context(tc.tile_pool(name="pdram", bufs=1, space="DRAM"))

        beta_d = pdram.tile([1, T_], F32)
        sums_loc_d = pdram.tile([1, TS_], F32)
        zbuf = pdram.tile([N_CORES, DD, TS_], F32)
        zred = pdram.tile([DD, TS_], F32)

        zsb = pz.tile([DD, T_], F32, tag="zsb")

        dh = [None] * 4
        dhl = [None] * 4
        bc_tiles = [None] * NJC
        ht_tiles = [None] * NRT
        negc = [None] * NRT

        for rep in range(nrep):
            first = rep == 0
            # ---------------- phase 0: DH, beta, prep ----------------
            with (
                tc.tile_pool(name="p0c", bufs=1) as p0c,
                tc.tile_pool(name="p0", bufs=3) as p0,
            ):
                h16 = p0c.tile([DD, T_], F16, tag="h16")
                nc.sync.dma_start(h16[:], io["h16"])
                dt16 = p0c.tile([DD, NN], F16, tag="dt16")
                nc.sync.dma_start(dt16[:], io["dt16"])
                hs16 = p0c.tile([DD, TS_], F16, tag="hs16")
                nc.sync.dma_start(hs16[:], io["hs16"])
                ones16 = p0c.tile([DD, 1], F16, tag="ones")
                nc.vector.memset(ones16[:], 1.0)

                for nt in range(4):
                    dh[nt] = pdh.tile([DD, T_], F16, tag="dh", name=f"dh{nt}")
                    dhl[nt] = pdhl.tile([DD, TS_], F16, tag="dhl", name=f"dhl{nt}")

                # full DH = D @ H, squared-column sums -> beta row in DRAM
                for jc in range(NJC):
                    sl = jc * 512
                    s = pZp.tile([DD, 512], F32, tag="z")
                    for nt in range(4):
                        g = pG.tile([DD, 512], F32, tag="g")
                        nc.tensor.matmul(
                            g[:],
                            lhsT=dt16[:, nt * 128 : (nt + 1) * 128],
                            rhs=h16[:, sl : sl + 512],
                            start=True,
                            stop=True,
                        )
                        nc.scalar.copy(dh[nt][:, sl : sl + 512], g[:])
                        sq = p0.tile([DD, 512], F16, tag="sq")
                        nc.vector.tensor_mul(
                            sq[:], dh[nt][:, sl : sl + 512], dh[nt][:, sl : sl + 512]
                        )
                        nc.tensor.matmul(
                            s[0:1, :],
                            lhsT=ones16[:],
                            rhs=sq[:],
                            start=(nt == 0),
                            stop=(nt == 3),
                        )
                    bsb = p0.tile([1, 512], F32, tag="bsb")
                    nc.scalar.activation(bsb[:], s[0:1, :], AF.Exp, bias=0.0, scale=-0.5)
                    nc.sync.dma_start(beta_d[0:1, sl : sl + 512], bsb[:])

                # local DH columns (own shard) + own-row sums -> c_t offsets
                sloc = [None] * NJCS
                for half in range(NJCS):
                    s = pZp.tile([DD, 512], F32, tag="z")
                    sloc[half] = s
                    for nt in range(4):
                        g = pG.tile([DD, 512], F32, tag="g")
                        nc.tensor.matmul(
                            g[:],
                            lhsT=dt16[:, nt * 128 : (nt + 1) * 128],
                            rhs=hs16[:, half * 512 : (half + 1) * 512],
                            start=True,
                            stop=True,
                        )
                        nc.scalar.copy(dhl[nt][:, half * 512 : (half + 1) * 512], g[:])
                        sq = p0.tile([DD, 512], F16, tag="sq")
                        nc.vector.tensor_mul(
                            sq[:],
                            dhl[nt][:, half * 512 : (half + 1) * 512],
                            dhl[nt][:, half * 512 : (half + 1) * 512],
                        )
                        nc.tensor.matmul(
                            s[0:1, :],
                            lhsT=ones16[:],
                            rhs=sq[:],
                            start=(nt == 0),
                            stop=(nt == 3),
                        )
                    nlsb = p0.tile([1, 512], F32, tag="nlsb")
                    nc.scalar.mul(nlsb[:], s[0:1, :], -0.5)
                    nc.sync.dma_start(
                        sums_loc_d[0:1, half * 512 : (half + 1) * 512], nlsb[:]
                    )

                sld = sums_loc_d[:]
                for rt in range(NRT):
                    negc[rt] = pnegc.tile([128, 1], F32, tag="negc", name=f"negc{rt}")
                    nc.gpsimd.dma_start(
                        out=negc[rt][:],
                        in_=bass.AP(
                            tensor=sld.tensor,
                            offset=sld.offset + rt * 128,
                            ap=[[1, 128], [0, 1]],
                        ),
                    )

                bd = beta_d[:]
                for jc in range(NJC):
                    bc_tiles[jc] = pbc.tile([128, 512], F32, tag="bc", name=f"bc{jc}")
                    nc.gpsimd.dma_start(
                        out=bc_tiles[jc][:],
                        in_=bass.AP(
                            tensor=bd.tensor,
                            offset=bd.offset + jc * 512,
                            ap=[[0, 128], [1, 512]],
                        ),
                    )

                for rt in range(NRT):
                    ht_tiles[rt] = pht.tile([128, DD], F16, tag="ht", name=f"ht{rt}")
                    nc.sync.dma_start(
                        ht_tiles[rt][:], io["ht16"][rt * 128 : (rt + 1) * 128, :]
                    )

            if stage == "p0":
                continue
            # ---------------- phase 1: fused attention sweep ----------------
            with (
                tc.tile_pool(name="pe", bufs=3) as pe,
                tc.tile_pool(name="pf", bufs=2 * NJC) as pf,
            ):
                f_tiles = {}
                hsc_tiles = {}

                def emit_g(rt):
                    rparts = psc.tile([128, NJC], F32, tag="rparts")
                    for jc in range(NJC):
                        sl = jc * 512
                        g = pG.tile([DD, 512], F32, tag="g")
                        for kt in range(4):
                            nc.tensor.matmul(
                                g[:],
                                lhsT=dhl[kt][:, rt * 128 : (rt + 1) * 128],
                                rhs=dh[kt][:, sl : sl + 512],
                                start=(kt == 0),
                                stop=(kt == 3),
                            )
                        e = pe.tile([128, 512], F32, tag="e")
                        nc.scalar.activation(
                            e[:], g[:], AF.Exp, bias=negc[rt][:], scale=1.0
                        )
                        f = pf.tile([128, 512], F16, tag="f")
                        nc.vector.scalar_tensor_tensor(
                            out=f[:],
                            in0=e[:],
                            scalar=1.0,
                            in1=bc_tiles[jc][:],
                            op0=OP.mult,
                            op1=OP.mult,
                            accum_out=rparts[:, jc : jc + 1],
                        )
                        f_tiles[(rt, jc)] = f
                    rtot = psc.tile([128, 1], F32, tag="rtot")
                    nc.vector.reduce_sum(rtot[:], rparts[:], axis=mybir.AxisListType.X)
                    rinv = psc.tile([128, 1], F32, tag="rinv")
                    nc.vector.reciprocal(rinv[:], rtot[:])
                    hsc = psc.tile([128, DD], F16, tag="hsc")
                    nc.vector.tensor_scalar(
                        out=hsc[:],
                        in0=ht_tiles[rt][:],
                        scalar1=rinv[:],
                        scalar2=None,
                        op0=OP.mult,
                    )
                    hsc_tiles[rt] = hsc

                def emit_z(rt):
                    for jc in range(NJC):
                        sl = jc * 512
                        z = pZp.tile([DD, 512], F32, tag="z")
                        nc.tensor.matmul(
                            z[:],
                            lhsT=hsc_tiles[rt][:],
                            rhs=f_tiles.pop((rt, jc))[:],
                            start=True,
                            stop=True,
                        )
                        if first and rt == 0:
                            nc.vector.tensor_copy(zsb[:, sl : sl + 512], z[:])
                        else:
                            nc.vector.tensor_add(
                                zsb[:, sl : sl + 512], z[:], zsb[:, sl : sl + 512]
                            )

                emit_g(0)
                for rt in range(1, NRT):
                    emit_g(rt)
                    emit_z(rt - 1)
                emit_z(NRT - 1)

        # ---------------- finale: reduce-scatter + LISTA update ----------------
        with tc.tile_pool(name="pfin", bufs=1) as pfin:
            if stage != "full":
                dbg = pfin.tile([DD, TS_], F32, tag="dbg")
                if stage == "p0":
                    nc.scalar.copy(dbg[:], dh[0][:, 0:TS_])
                else:
                    nc.vector.tensor_copy(dbg[:], zsb[:, 0:TS_])
                nc.sync.dma_start(io["y"][:], dbg[:])
                return
            for b in range(N_CORES):
                nc.sync.dma_start(zbuf[b, :, :], zsb[:, b * TS_ : (b + 1) * TS_])
            nc.gpsimd.collective_compute(
                "ReduceScatter",
                OP.add,
                replica_groups=[list(range(N_CORES))],
                ins=[zbuf[:]],
                outs=[zred[:]],
            )
            zs2 = pfin.tile([DD, TS_], F32, tag="zs2")
            nc.sync.dma_start(zs2[:], zred[:])
            nthr = pfin.tile([DD, 1], F32, tag="nthr")
            nc.vector.memset(nthr[:], -thres)
            ut = pfin.tile([DD, DD], F32, tag="ut")
            nc.sync.dma_start(ut[:], io["ut"])
            vt = [pfin.tile([128, DD], F32, tag=f"vt{k}", name=f"vt{k}") for k in range(3)]
            xs = [pfin.tile([128, TS_], F32, tag=f"xs{k}", name=f"xs{k}") for k in range(3)]
            for k in range(3):
                nc.sync.dma_start(vt[k][:], io["vt"][k * 128 : (k + 1) * 128, :])
                nc.sync.dma_start(xs[k][:], io["xs"][k * 128 : (k + 1) * 128, :])
            for jc in range(NJCS):
                sl = jc * 512
                mat = pZp.tile([DD, 512], F32, tag="z")
                nc.tensor.matmul(
                    mat[:], lhsT=ut[:], rhs=zs2[:, sl : sl + 512], start=True, stop=False
                )
                for k in range(3):
                    nc.tensor.matmul(
                        mat[:],
                        lhsT=vt[k][:],
                        rhs=xs[k][:, sl : sl + 512],
                        start=False,
                        stop=(k == 2),
                    )
                pos = pfin.tile([DD, 512], F32, tag="pos")
                nc.scalar.activation(pos[:], mat[:], AF.Relu, bias=nthr[:], scale=1.0)
                neg = pfin.tile([DD, 512], F32, tag="neg")
                nc.scalar.activation(neg[:], mat[:], AF.Relu, bias=nthr[:], scale=-1.0)
                outsb = pfin.tile([DD, 512], F32, tag="outsb")
                nc.vector.tensor_sub(outsb[:], pos[:], neg[:])
                nc.sync.dma_start(io["y"][:, sl : sl + 512], outsb[:])


def build(thres, nrep=1, T_=T, debug=False, stage="full"):
    nc = bacc.Bacc(
        "TRN2",
        target_bir_lowering=False,
        debug=debug,
        num_devices=N_CORES,
    )
    TS_ = T_ // N_CORES
    io = {
        "h16": nc.dram_tensor("h16", [DD, T_], F16, kind="ExternalInput").ap(),
        "dt16": nc.dram_tensor("dt16", [DD, NN], F16, kind="ExternalInput").ap(),
        "hs16": nc.dram_tensor("hs16", [DD, TS_], F16, kind="ExternalInput").ap(),
        "ht16": nc.dram_tensor("ht16", [TS_, DD], F16, kind="ExternalInput").ap(),
        "xs": nc.dram_tensor("xs", [MM, TS_], F32, kind="ExternalInput").ap(),
        "ut": nc.dram_tensor("ut", [DD, DD], F32, kind="ExternalInput").ap(),
        "vt": nc.dram_tensor("vt", [MM, DD], F32, kind="ExternalInput").ap(),
        "y": nc.dram_tensor("y", [DD, TS_], F32, kind="ExternalOutput").ap(),
    }
    with tile.TileContext(nc) as tc:
        _emit(nc, tc, io, thres, nrep, T_=T_, stage=stage)
    nc.compile()
    return nc


def prep_inputs(H, D, X, U, V, l2f):
    """Host-side layout prep: casts, transposes, per-core slices."""
    H = np.asarray(H, np.float32)
    D = np.asarray(D, np.float32)
    X = np.asarray(X, np.float32)
    U = np.asarray(U, np.float32)
    V = np.asarray(V, np.float32)
    h16 = H.astype(np.float16)
    dt16 = np.ascontiguousarray(D.T).astype(np.float16)
    ut = np.ascontiguousarray((l2f * U).T)
    vt = np.ascontiguousarray(V.T)
    T_ = H.shape[1]
    TS_ = T_ // N_CORES
    in_maps = []
    for m in range(N_CORES):
        sh = slice(m * TS_, (m + 1) * TS_)
        in_maps.append(
            {
                "h16": h16,
                "dt16": dt16,
                "hs16": np.ascontiguousarray(h16[:, sh]),
                "ht16": np.ascontiguousarray(H[:, sh].T).astype(np.float16),
                "xs": np.ascontiguousarray(X[:, sh]),
                "ut": ut,
                "vt": vt,
            }
        )
    return in_maps


_RUNNER_CACHE = {}


def _get_runner(thres, nrep=1):
    """Build + compile once, return a cached callable(in_maps) -> list of {y: ...}."""
    key = (float(thres), nrep)
    if key in _RUNNER_CACHE:
        return _RUNNER_CACHE[key]

    nc = build(float(thres), nrep=nrep)

    import jax
    from jax.sharding import Mesh, PartitionSpec
    from concourse import bass2jax
    from concourse.bass2jax import _bass_exec_p, partition_id_tensor

    bass2jax.install_neuronx_cc_hook()

    in_names = []
    out_names = []
    out_avals = []
    zero_shapes = []
    partition_name = nc.partition_id_tensor.name if nc.partition_id_tensor else None
    for alloc in nc.m.functions[0].allocations:
        if not isinstance(alloc, mybir.MemoryLocationSet):
            continue
        name = alloc.memorylocations[0].name
        if alloc.kind == "ExternalInput":
            if name != partition_name:
                in_names.append(name)
        elif alloc.kind == "ExternalOutput":
            shape = list(alloc.tensor_shape)
            np_dt = mybir.dt.np(alloc.dtype)
            out_names.append(name)
            out_avals.append(jax.core.ShapedArray(shape, np_dt))
            zero_shapes.append((shape, np_dt))

    n_params = len(in_names)
    n_outs = len(out_names)
    all_in_names = list(in_names) + list(out_names)
    if partition_name is not None:
        all_in_names.append(partition_name)
    donate = tuple(range(n_params, n_params + n_outs))

    def _body(*args):
        operands = list(args)
        if partition_name is not None:
            operands.append(partition_id_tensor())
        outs = _bass_exec_p.bind(
            *operands,
            out_avals=tuple(out_avals),
            in_names=tuple(all_in_names),
            out_names=tuple(out_names),
            lowering_input_output_aliases=(),
            sim_require_finite=True,
            sim_require_nnan=True,
            nc=nc,
        )
        return tuple(outs)

    from jax.experimental.shard_map import shard_map

    devices = jax.devices()[:N_CORES]
    mesh = Mesh(np.asarray(devices), ("core",))
    in_specs = (PartitionSpec("core"),) * (n_params + n_outs)
    out_specs = (PartitionSpec("core"),) * n_outs
    sharded = jax.jit(
        shard_map(
            _body, mesh=mesh, in_specs=in_specs, out_specs=out_specs, check_rep=False
        ),
        donate_argnums=donate,
        keep_unused=True,
    )

    def run(in_maps):
        per_core = [[np.asarray(m[name]) for name in in_names] for m in in_maps]
        concat_in = [
            np.concatenate([per_core[c][i] for c in range(N_CORES)], axis=0)
            for i in range(n_params)
        ]
        concat_zeros = [
            np.zeros((N_CORES * s[0], *s[1:]), dt) for (s, dt) in zero_shapes
        ]
        out_arrs = sharded(*concat_in, *concat_zeros)
        return [
            {
                name: np.asarray(out_arrs[i]).reshape(
                    N_CORES, *zero_shapes[i][0]
                )[c]
                for i, name in enumerate(out_names)
            }
            for c in range(N_CORES)
        ]

    _RUNNER_CACHE[key] = run
    return run


def kernel(H, D, X, U, V, l1, l2, c):
    l2f = float(np.asarray(l2))
    thres = float(np.asarray(l1)) / 1.0  # C_INIT = 1.0, forward arg c unused
    in_maps = prep_inputs(H, D, X, U, V, l2f)
    run = _get_runner(thres, nrep=1)
    results = run(in_maps)
    out = np.concatenate([results[m]["y"] for m in range(N_CORES)], axis=1)
    return out.astype(np.float32)
